# revision 18
# baseline (speedup 1.0000x reference)
"""AttnBlock3D (GroupNorm + single-head self-attention + proj + residual) on 8 trn2 cores.

Sharding: core i handles (batch b = i//4, query-block qb = i%4) of 1024 query
positions. Attention is permutation-equivariant over positions, so each core
receives its batch's x with the position axis rolled so that its query block
occupies columns 0:1024. Each core computes GroupNorm + full V for its batch
(4x replicated within a batch group) and attention/proj/residual for its own
1024 query positions. No collectives.

Algebraic restructures (exact up to fp rounding):
  * Q and K projections are never materialized. With Wqk = Wk^T Wq and
    bqk = Wk^T bq (host-computed),
      scores^T[nk, nq] = xn[:, nk] . (Wqk xn[:, :1024] + bqk)[:, nq]
                         + (per-nq constants, which cancel in softmax).
  * Softmax skips the max subtraction; normalization is deferred and folded
    into the o eviction (o8 = o_psum * 256/rowsum), so the epilogue is just
    proj -> one fused (ps/4096 + xq) scalar_tensor_tensor + DMA.
  * The rowsum matmul uses an all-1/16 [128,2,128] stationary so the
    per-query sum lands broadcast across all 128 PSUM partitions; the
    reciprocal runs as a full-width [128,512] reciprocal_approx_fast
    (DVE InstReciprocal costs 4us; the approx is ~0.8us at 18 bits).
  * The residual base (x + folded proj/v bias) is precomputed ON THE HOST
    and shipped as a bf16 input, so no head DVE op touches it.
  * gn_weight is folded into the host-built selbc broadcast matrix.

fp8 DoubleRow everywhere: B, V, scores, AV+rowsum, proj all run as fp8e4
DoubleRow (2 fp8 MACs/cell/cycle), operands are [128, 2, free] planar
slices. Wqk/Wv/Wp are prescaled x16 on the host (fp8 subnormal avoidance);
exp gets scale SCALE/16 and bias -ln16; V keeps its x16 (cancels against
p8's 1/16 in AV exactly); the residual add's 1/4096 scalar absorbs
Wp's 16 and o8's 256/rowsum normalization. All exact in fp.

Engine assignment: ScalarE runs ONLY Sqrt (GN, once) and Exp (64 tiles; the
Exp ACT_TABLE_LOAD is prefetched by a dummy exp right after the Sqrt). ALL
PSUM evictions run on the DVE, as wide as PSUM allows: V evicts 4 banks
(2048) at a time from a [128,4,512] PSUM tile, B evicts 2 banks. V and B
accumulate in ps_v/ps_work so the scores stream's PSUM rotation never waits
on a V eviction. GpSimd issues every non-x DMA so the two HW queues carry
only x during the head.
"""

import math

import numpy as np
import ml_dtypes

import concourse.bass as bass
import concourse.tile as tile
from concourse import bacc, mybir
from concourse.bass import ds, ts
from concourse.bass_utils import run_bass_kernel_spmd

B, C, H, W, D = 2, 512, 16, 16, 16
N = H * W * D              # 4096 positions
NQ = N // 4                # 1024 query positions per core
T = C // 128               # 4 channel tiles
NKT = N // 128             # 32 key tiles
NKP = NKT // 2             # 16 key-tile pairs (DoubleRow granularity)
NQC = NQ // 512            # 2 query chunks of 512
GROUPS = 32
GSIZE = C // GROUPS        # 16 channels per group
EPS = 1e-6
SCALE = float(C) ** -0.5
WS = 16.0                  # host prescale on Wqk / Wv / Wp
ESC = SCALE / WS           # exp scale (absorbs Wqk's x16)
EBIAS = -math.log(16.0)    # exp bias: p < ~15; cancels via rowsum
RSONE = 1.0 / 4.0          # rowsum stationary -> rsinv = 64/rowsum
PRSC = 1.0 / 1024.0        # epilogue scalar: (16Wp)(16V)(64/rs) -> /1024

F32 = mybir.dt.float32
F16 = mybir.dt.float16
BF16 = mybir.dt.bfloat16
F8 = mybir.dt.float8e4
DR = mybir.MatmulPerfMode.DoubleRow
MUL = mybir.AluOpType.mult
ADD = mybir.AluOpType.add


def build_nc(reps: int = 1):
    nc = bacc.Bacc("TRN2", target_bir_lowering=False)

    env = {}
    env["x_d"] = nc.dram_tensor("x", [C, N], BF16, kind="ExternalInput")
    env["xq_d"] = nc.dram_tensor("xq", [C, NQ], BF16, kind="ExternalInput")
    env["wqkT_d"] = nc.dram_tensor("wqkT", [C, C], F8, kind="ExternalInput")
    env["wvT_d"] = nc.dram_tensor("wvT", [C, C], F8, kind="ExternalInput")
    env["wpT_d"] = nc.dram_tensor("wpT", [C, C], F8, kind="ExternalInput")
    env["bqk_d"] = nc.dram_tensor("bqk", [128, T], F32, kind="ExternalInput")
    env["gnb_d"] = nc.dram_tensor("gnb", [128, T], F32, kind="ExternalInput")
    env["selred_d"] = nc.dram_tensor("selred", [128, T, GROUPS], F32, kind="ExternalInput")
    env["selbc_d"] = nc.dram_tensor("selbc", [GROUPS, C], F32, kind="ExternalInput")
    env["out_d"] = nc.dram_tensor("out", [C, NQ], F32, kind="ExternalOutput")

    with tile.TileContext(nc) as tc:
        import contextlib

        with contextlib.ExitStack() as ctx:
            env["const"] = ctx.enter_context(tc.tile_pool(name="const", bufs=1))
            env["big"] = ctx.enter_context(tc.tile_pool(name="big", bufs=1))
            env["mid"] = ctx.enter_context(tc.tile_pool(name="mid", bufs=1))
            env["stats"] = ctx.enter_context(tc.tile_pool(name="stats", bufs=2))
            env["small"] = ctx.enter_context(tc.tile_pool(name="small", bufs=2))
            env["ppool"] = ctx.enter_context(tc.tile_pool(name="ppool", bufs=2))
            env["ps_work"] = ctx.enter_context(tc.tile_pool(name="ps_work", bufs=1, space="PSUM"))
            env["ps_v"] = ctx.enter_context(tc.tile_pool(name="ps_v", bufs=1, space="PSUM"))
            env["ps_rs"] = ctx.enter_context(tc.tile_pool(name="ps_rs", bufs=1, space="PSUM"))
            env["ps_small"] = ctx.enter_context(tc.tile_pool(name="ps_small", bufs=1, space="PSUM"))

            const = env["const"]
            ones8b = const.tile([128, 2, 128], F8, tag="ones8b")
            nc.vector.memset(ones8b, RSONE)
            env["ones8b"] = ones8b
            epst = const.tile([GROUPS, 1], F32, tag="epst")
            nc.vector.memset(epst, EPS)
            env["epst"] = epst
            ebias = const.tile([128, 1], F32, tag="ebias")
            nc.vector.memset(ebias, EBIAS)
            env["ebias"] = ebias
            dummy = const.tile([128, 1], F32, tag="dummy")
            env["dummy"] = dummy
            idf32 = const.tile([128, 16], F32, tag="idf32")
            nc.vector.memset(idf32, 1.0)
            env["idf32"] = idf32

            for rep in range(reps):
                body(nc, tc, env, first=(rep == 0))

    nc.compile()
    return nc


def body(nc, tc, env, first=True):
    big, mid, stats, small, ppool = (env[k] for k in ("big", "mid", "stats", "small", "ppool"))
    ps_work, ps_v, ps_rs, ps_small = (env[k] for k in ("ps_work", "ps_v", "ps_rs", "ps_small"))
    x_d, out_d = env["x_d"], env["out_d"]
    const = env["const"]
    ones8b, epst, ebias, dummy, idf32 = (
        env[k] for k in ("ones8b", "epst", "ebias", "dummy", "idf32"))
    AF = mybir.ActivationFunctionType
    wu_count = [0]

    def warmups(n, t_avail, lo=0, width=N):
        # full-array dummy bf16 matmuls on already-landed x pieces: 128-col
        # stationary + 256-col moving keeps all PE row/col groups active so
        # the HAM clock gate sees real occupancy, not 1-column trickles.
        for _ in range(n):
            i = wu_count[0]
            wu_count[0] += 1
            wu_ps = ps_rs.tile([128, 256], F32, tag="psrs", name=f"wu{i}")
            mo = lo + 128 + (i * 256) % (width - 384)
            nc.tensor.matmul(wu_ps, x_sb[:, t_avail, ds(lo, 128)],
                             x_sb[:, t_avail, ds(mo, 256)],
                             start=True, stop=True)

    def wu_paced(src):
        # small f32 matmul reading a just-produced DVE result: lands in the
        # PE queue right when the DVE finishes it, so sparse PE activity
        # tracks DVE progress through the stats phase (maintains un-throttle)
        i = wu_count[0]
        wu_count[0] += 1
        fw = src.shape[-1]
        wu_ps = ps_rs.tile([16, fw], F32, tag="psrs", name=f"wp{i}")
        nc.tensor.matmul(wu_ps, idf32[:src.shape[0], :], src,
                         start=True, stop=True)

    if first:
        def load_small_consts():
            for nm in ("bqk", "gnb"):
                sb = const.tile([128, T], F32, tag=nm, name=f"sb_{nm}")
                nc.gpsimd.dma_start(out=sb, in_=env[f"{nm}_d"][:, :])
                env[nm] = sb
            selred = const.tile([128, T, GROUPS], F32, tag="selred")
            nc.gpsimd.dma_start(out=selred, in_=env["selred_d"][:, :, :])
            env["selred"] = selred
            selbc = const.tile([GROUPS, C], F32, tag="selbc")
            nc.gpsimd.dma_start(out=selbc, in_=env["selbc_d"][:, :])
            env["selbc"] = selbc

        def load_weights():
            for nm in ("wqkT", "wvT", "wpT"):
                sb = const.tile([128, T, C], F8, tag=nm, name=f"sb_{nm}")
                dr_ = env[f"{nm}_d"]
                for t in range(T):
                    nc.gpsimd.dma_start(out=sb[:, t, :], in_=dr_[ts(t, 128), :])
                env[nm] = sb
            xq16 = mid.tile([128, T, NQ], BF16, tag="xq16")
            for t in range(T):
                nc.gpsimd.dma_start(out=xq16[:, t, :],
                                    in_=env["xq_d"][ts(t, 128), :])
            env["xq16"] = xq16

    # -------- load x + GroupNorm stats, pipelined per piece --------
    # x rides the two HW queues exclusively (everything else issues from
    # GpSimd). Tile halves alternate queues; the very first half-tile lands
    # in 512-col pieces so bn_stats starts ~3us earlier.
    x_sb = big.tile([128, T, N], BF16, tag="x")
    sts = []
    for t in range(T):
        st = stats.tile([128, 8, 6], F32, tag=f"bnstats{t}", bufs=1, name=f"st{t}")
        sts.append(st)

    psg = ps_small.tile([GROUPS, 2], F32, tag="pssmall")
    for t in range(T):
        for h in range(2):
            eng = nc.sync if h == 0 else nc.scalar
            sub = 4 if (t, h) == (0, 0) else 1
            for p in range(sub):
                w = 2048 // sub
                lo = h * 2048 + p * w
                eng.dma_start(out=x_sb[:, t, ds(lo, w)],
                              in_=x_d[ts(t, 128), ds(lo, w)])
                if (t, h, p) == (0, 0, 0):
                    warmups(16, 0, lo=0, width=512)
                for s in range(w // 512):
                    nc.vector.bn_stats(out=sts[t][:, 4 * h + p * (w // 512) + s, :],
                                       in_=x_sb[:, t, ds(lo + s * 512, 512)])
                    wu_paced(sts[t][:, 4 * h + p * (w // 512) + s, :])
        if t == 0 and first:
            load_small_consts()
        mv = stats.tile([128, 2], F32, tag=f"mv{t}", bufs=1, name=f"mv{t}")
        nc.vector.bn_aggr(out=mv, in_=sts[t])
        # mv := (mean, E[x^2]) ; E[x^2] = var + mean^2
        msq = stats.tile([128, 1], F32, tag="msq")
        nc.vector.tensor_mul(msq, mv[:, 0:1], mv[:, 0:1])
        nc.vector.tensor_add(mv[:, 1:2], mv[:, 1:2], msq)
        wu_paced(mv)
        nc.tensor.matmul(psg, env["selred"][:, t, :], mv,
                         start=(t == 0), stop=(t == T - 1))
    if first:
        load_weights()
    warmups(4, T - 1)

    wqkT, wvT, wpT = env["wqkT"], env["wvT"], env["wpT"]
    bqk, gnb, selbc, xq16 = env["bqk"], env["gnb"], env["selbc"], env["xq16"]

    # group scale/offset straight off the psg PSUM: rstd = 1/sqrt(var+eps),
    # offset = -mean*rstd  (gn_weight is folded into selbc host-side)
    psgs = small.tile([GROUPS, 2], F32, tag="psgs", bufs=1)
    nc.vector.tensor_copy(psgs, psg)
    gsc = small.tile([GROUPS, 2], F32, tag="gsc", bufs=1)
    gtmp = small.tile([GROUPS, 2], F32, tag="gtmp", bufs=1)
    nc.vector.tensor_mul(gtmp[:, 0:1], psgs[:, 0:1], psgs[:, 0:1])      # mean^2
    nc.vector.tensor_sub(gtmp[:, 1:2], psgs[:, 1:2], gtmp[:, 0:1])      # var
    wu_paced(gtmp)
    nc.scalar.activation(out=gsc[:, 0:1], in_=gtmp[:, 1:2], func=AF.Sqrt, bias=epst)
    # preload the Exp activation table NOW (ScalarE idle; its next real use
    # is the first scores exp, which must not eat the 1.5us table load)
    nc.scalar.activation(out=dummy, in_=ebias, func=AF.Exp, bias=0.0)
    nc.vector.reciprocal(gsc[:, 0:1], gsc[:, 0:1])                      # rstd
    nc.vector.tensor_mul(gsc[:, 1:2], psgs[:, 0:1], gsc[:, 0:1])
    nc.vector.tensor_scalar_mul(gsc[:, 1:2], gsc[:, 1:2], -1.0)        # offset
    wu_paced(gsc)

    # per-channel (scale, offset) via the gnw-folded broadcast matmul, then
    # GN-apply -> xn (fp8e4) on the DVE in 2048 pieces, query-half first
    scof = small.tile([128, T, 2], F32, tag="scof", bufs=1)
    xn = mid.tile([128, T, N], F8, tag="xn")
    for t in range(T):
        psbc = ps_small.tile([128, 2], F32, tag="pssmall", name=f"psbc{t}")
        nc.tensor.matmul(psbc, selbc[:, ts(t, 128)], gsc, start=True, stop=True)
        nc.vector.tensor_copy(scof[:, t, 0:1], psbc[:, 0:1])
        nc.vector.tensor_scalar_add(scof[:, t, 1:2], psbc[:, 1:2],
                                    gnb[:, t:t + 1])
    for h in range(2):
        for t in range(T):
            nc.vector.tensor_scalar(
                out=xn[:, t, ds(h * 2048, 2048)], in0=x_sb[:, t, ds(h * 2048, 2048)],
                scalar1=scof[:, t, 0:1], scalar2=scof[:, t, 1:2],
                op0=MUL, op1=ADD,
            )
    warmups(4, T - 1)

    # -------- B = Wqk xn_q + bqk  (fp8 DR; 2-bank PSUM, single wide evict) --
    b_sb = mid.tile([128, T, NQC, 512], F8, tag="b")
    for t_out in range(T):
        bps = ps_work.tile([128, 2, 512], F32, tag="pswork", name=f"bps{t_out}")
        for nch in range(NQC):
            for g in range(T // 2):
                nc.tensor.matmul(bps[:, nch, :],
                                 wqkT[:, 2 * g:2 * g + 2, ts(t_out, 128)],
                                 xn[:, 2 * g:2 * g + 2, ds(nch * 512, 512)],
                                 start=(g == 0), stop=(g == T // 2 - 1),
                                 perf_mode=DR)
        nc.vector.tensor_scalar_add(b_sb[:, t_out, :, :], bps[:, :, :],
                                    bqk[:, t_out:t_out + 1])

    # -------- V^T (fp8 DR; 4-bank PSUM in ps_v, single 2048-wide evict) ----
    # keeps Wv's x16: v8 = 16*v, cancels against p8's 1/16 in AV exactly
    vT = big.tile([128, NKT, C], F8, tag="vT")
    for vb in range(NKT // 4):
        vps = ps_v.tile([128, 4, 512], F32, tag="psv", name=f"vps{vb}")
        for k in range(4):
            nkt = 4 * vb + k
            for g in range(T // 2):
                nc.tensor.matmul(vps[:, k, :], xn[:, 2 * g:2 * g + 2, ts(nkt, 128)],
                                 wvT[:, 2 * g:2 * g + 2, :],
                                 start=(g == 0), stop=(g == T // 2 - 1),
                                 perf_mode=DR)
        nc.vector.tensor_copy(vT[:, 4 * vb:4 * vb + 4, :], vps[:, :, :])

    # -------- attention + proj per query chunk --------
    xq32 = mid.tile([128, T, NQ], F32, tag="xq32")

    def pe_epilogue(ch):
        # proj (fp8 DR) ping-pongs the ps_small/ps_rs banks; the fused
        # scalar_tensor_tensor adds the host-precomputed residual base
        for t_out in range(T):
            pool = ps_small if t_out % 2 == 0 else ps_rs
            ps = pool.tile([128, 512], F32,
                           tag="pssmall" if t_out % 2 == 0 else "psrs",
                           name=f"prps{ch}_{t_out}")
            for g in range(T // 2):
                nc.tensor.matmul(ps, wpT[:, 2 * g:2 * g + 2, ts(t_out, 128)],
                                 o_sb[:, 2 * g:2 * g + 2, ds(ch * 512, 512)],
                                 start=(g == 0), stop=(g == T // 2 - 1),
                                 perf_mode=DR)
            nc.vector.scalar_tensor_tensor(
                out=xq32[:, t_out, ds(ch * 512, 512)], in0=ps, scalar=PRSC,
                in1=xq16[:, t_out, ds(ch * 512, 512)], op0=MUL, op1=ADD)
            nc.sync.dma_start(out=out_d[ts(t_out, 128), ds(ch * 512, 512)],
                              in_=xq32[:, t_out, ds(ch * 512, 512)])

    o_sb = mid.tile([128, T, NQ], F8, tag="o")
    for ch in range(NQC):
        if ch > 0:
            pe_epilogue(ch - 1)
        o_ps = ps_v.tile([128, 4, 512], F32, tag="psv", name=f"ops{ch}")
        rs_ps = ps_rs.tile([128, 512], F32, tag="psrs", name=f"rs{ch}")
        p8 = ppool.tile([128, NKT, 512], F8, tag="p")
        s_pair = ps_work.tile([128, 2, 512], F32, tag="pswork", name=f"sp{ch}")

        def emit_av(j):
            nc.tensor.matmul(rs_ps, ones8b, p8[:, 2 * j:2 * j + 2, :],
                             start=(j == 0), stop=(j == NKP - 1), perf_mode=DR)
            for tc_in in range(T):
                nc.tensor.matmul(o_ps[:, tc_in, :],
                                 vT[:, 2 * j:2 * j + 2, ts(tc_in, 128)],
                                 p8[:, 2 * j:2 * j + 2, :],
                                 start=(j == 0), stop=(j == NKP - 1), perf_mode=DR)

        prev = None
        for j in range(NKP):
            for h in range(2):
                nkt = 2 * j + h
                for g in range(T // 2):
                    nc.tensor.matmul(s_pair[:, h, :], xn[:, 2 * g:2 * g + 2, ts(nkt, 128)],
                                     b_sb[:, 2 * g:2 * g + 2, ch, :],
                                     start=(g == 0), stop=(g == T // 2 - 1),
                                     perf_mode=DR)
                nc.scalar.activation(out=p8[:, nkt, :], in_=s_pair[:, h, :],
                                     func=AF.Exp, scale=ESC, bias=ebias)
            if prev is not None:
                emit_av(prev)
            prev = j
        emit_av(prev)

        # rsinv = 256/rowsum (fast approx, ~18 bits); normalize-at-eviction
        # o8 = o_psum * rsinv ~ N(0, 6.7) — frees the accumulators for the
        # next chunk and leaves the epilogue a pure proj+add.
        rsinv = small.tile([128, 512], F32, tag="rsinv", name=f"rsinv{ch}")
        nc.vector.reciprocal_approx_fast(out=rsinv, in_=rs_ps)
        for tc_in in range(T):
            nc.vector.tensor_mul(o_sb[:, tc_in, ds(ch * 512, 512)],
                                 o_ps[:, tc_in, :], rsinv)

    pe_epilogue(NQC - 1)


_NC_CACHE = {}


def _get_nc(reps: int = 1):
    if reps not in _NC_CACHE:
        _NC_CACHE[reps] = build_nc(reps)
    return _NC_CACHE[reps]


def make_in_maps(x, gn_weight, gn_bias, qkv_weight, qkv_bias, proj_weight, proj_bias):
    x = np.asarray(x, np.float32)
    qkv_weight = np.asarray(qkv_weight, np.float32)
    proj_weight = np.asarray(proj_weight, np.float32)
    qkv_bias = np.asarray(qkv_bias, np.float32)
    proj_bias = np.asarray(proj_bias, np.float32)
    gn_weight = np.asarray(gn_weight, np.float32)
    gn_bias = np.asarray(gn_bias, np.float32)

    Wq, Wk, Wv = qkv_weight[0:C], qkv_weight[C:2 * C], qkv_weight[2 * C:3 * C]
    wqkT = np.ascontiguousarray((WS * (Wq.T @ Wk)).astype(ml_dtypes.float8_e4m3))
    wvT = np.ascontiguousarray((WS * Wv.T).astype(ml_dtypes.float8_e4m3))
    wpT = np.ascontiguousarray((WS * proj_weight.T).astype(ml_dtypes.float8_e4m3))

    def cols(v):  # [C] -> [128, T]
        return np.ascontiguousarray(v.reshape(T, 128).T.astype(np.float32))

    bqkv = WS * (Wk.T @ qkv_bias[0:C])
    fbv = proj_weight @ qkv_bias[2 * C:3 * C] + proj_bias

    p_idx = np.arange(128)
    selred = np.zeros((128, T, GROUPS), np.float32)
    selbc = np.zeros((GROUPS, C), np.float32)
    for t in range(T):
        g = t * (128 // GSIZE) + p_idx // GSIZE
        selred[p_idx, t, g] = 1.0 / GSIZE
        selbc[g, t * 128 + p_idx] = gn_weight[t * 128 + p_idx]

    shared = {
        "wqkT": wqkT, "wvT": wvT, "wpT": wpT,
        "bqk": cols(bqkv), "gnb": cols(gn_bias),
        "selred": selred, "selbc": selbc,
    }
    in_maps = []
    for core in range(8):
        b, qb = core // 4, core % 4
        xb = x[b].reshape(C, N)
        xr = np.roll(xb, -qb * NQ, axis=1)
        m = dict(shared)
        m["x"] = np.ascontiguousarray(xr.astype(ml_dtypes.bfloat16))
        m["xq"] = np.ascontiguousarray(
            (xr[:, 0:NQ] + fbv[:, None]).astype(ml_dtypes.bfloat16))
        in_maps.append(m)
    return in_maps


def kernel(x, gn_weight, gn_bias, qkv_weight, qkv_bias, proj_weight, proj_bias):
    nc = _get_nc(1)
    in_maps = make_in_maps(x, gn_weight, gn_bias, qkv_weight, qkv_bias,
                           proj_weight, proj_bias)
    res = run_bass_kernel_spmd(nc, in_maps, core_ids=list(range(8)))
    out = np.empty((B, C, N), np.float32)
    for core in range(8):
        b, qb = core // 4, core % 4
        out[b][:, qb * NQ:(qb + 1) * NQ] = res.results[core]["out"]
    return out.reshape(B, C, H, W, D)


# revision 19
# speedup vs baseline: 1.5478x; 1.5478x over previous
"""AttnBlock3D (GroupNorm + single-head self-attention + proj + residual) on 8 trn2 cores.

Sharding: core i handles (batch b = i//4, query-block qb = i%4) of 1024 query
positions. Attention is permutation-equivariant over positions, so each core
receives its batch's x with the position axis rolled so that its query block
occupies columns 0:1024. Each core computes GroupNorm + full V for its batch
(4x replicated within a batch group) and attention/proj/residual for its own
1024 query positions. No collectives.

Algebraic restructures (exact up to fp rounding):
  * Q and K projections are never materialized. With Wqk = Wk^T Wq and
    bqk = Wk^T bq (host-computed),
      scores^T[nk, nq] = xn[:, nk] . (Wqk xn[:, :1024] + bqk)[:, nq]
                         + (per-nq constants, which cancel in softmax).
  * Softmax skips the max subtraction; normalization is deferred and folded
    into the o eviction (o8 = o_psum * 256/rowsum), so the epilogue is just
    proj -> one fused (ps/4096 + xq) scalar_tensor_tensor + DMA.
  * The rowsum matmul uses an all-1/16 [128,2,128] stationary so the
    per-query sum lands broadcast across all 128 PSUM partitions; the
    reciprocal runs as a full-width [128,512] reciprocal_approx_fast
    (DVE InstReciprocal costs 4us; the approx is ~0.8us at 18 bits).
  * The residual base (x + folded proj/v bias) is precomputed ON THE HOST
    and shipped as a bf16 input, so no head DVE op touches it.
  * gn_weight is folded into the host-built selbc broadcast matrix.

fp8 DoubleRow everywhere: B, V, scores, AV+rowsum, proj all run as fp8e4
DoubleRow (2 fp8 MACs/cell/cycle), operands are [128, 2, free] planar
slices. Wqk/Wv/Wp are prescaled x16 on the host (fp8 subnormal avoidance);
exp gets scale SCALE/16 and bias -ln16; V keeps its x16 (cancels against
p8's 1/16 in AV exactly); the residual add's 1/4096 scalar absorbs
Wp's 16 and o8's 256/rowsum normalization. All exact in fp.

Engine assignment: ScalarE runs ONLY Sqrt (GN, once) and Exp (64 tiles; the
Exp ACT_TABLE_LOAD is prefetched by a dummy exp right after the Sqrt). ALL
PSUM evictions run on the DVE, as wide as PSUM allows: V evicts 4 banks
(2048) at a time from a [128,4,512] PSUM tile, B evicts 2 banks. V and B
accumulate in ps_v/ps_work so the scores stream's PSUM rotation never waits
on a V eviction. GpSimd issues every non-x DMA so the two HW queues carry
only x during the head.
"""

import math

import numpy as np
import ml_dtypes

import concourse.bass as bass
import concourse.tile as tile
from concourse import bacc, mybir
from concourse.bass import ds, ts
from concourse.bass_utils import run_bass_kernel_spmd

B, C, H, W, D = 2, 512, 16, 16, 16
N = H * W * D              # 4096 positions
NQ = N // 4                # 1024 query positions per core
T = C // 128               # 4 channel tiles
NKT = N // 128             # 32 key tiles
NKP = NKT // 2             # 16 key-tile pairs (DoubleRow granularity)
NQC = NQ // 512            # 2 query chunks of 512
GROUPS = 32
GSIZE = C // GROUPS        # 16 channels per group
EPS = 1e-6
SCALE = float(C) ** -0.5
WS = 16.0                  # host prescale on Wqk / Wv / Wp
ESC = SCALE / WS           # exp scale (absorbs Wqk's x16)
EBIAS = -math.log(16.0)    # exp bias: p < ~15; cancels via rowsum
RSONE = 1.0 / 4.0          # rowsum stationary -> rsinv = 64/rowsum
PRSC = 1.0 / 1024.0        # epilogue scalar: (16Wp)(16V)(64/rs) -> /1024

F32 = mybir.dt.float32
F16 = mybir.dt.float16
BF16 = mybir.dt.bfloat16
F8 = mybir.dt.float8e4
DR = mybir.MatmulPerfMode.DoubleRow
MUL = mybir.AluOpType.mult
ADD = mybir.AluOpType.add


def build_nc(reps: int = 1):
    nc = bacc.Bacc("TRN2", target_bir_lowering=False)

    env = {}
    env["x_d"] = nc.dram_tensor("x", [C, N], BF16, kind="ExternalInput")
    env["xq_d"] = nc.dram_tensor("xq", [C, NQ], BF16, kind="ExternalInput")
    env["wqkT_d"] = nc.dram_tensor("wqkT", [C, C], F8, kind="ExternalInput")
    env["wvT_d"] = nc.dram_tensor("wvT", [C, C], F8, kind="ExternalInput")
    env["wpT_d"] = nc.dram_tensor("wpT", [C, C], F8, kind="ExternalInput")
    env["bqk_d"] = nc.dram_tensor("bqk", [128, T], F32, kind="ExternalInput")
    env["gnb_d"] = nc.dram_tensor("gnb", [128, T], F32, kind="ExternalInput")
    env["selred_d"] = nc.dram_tensor("selred", [128, T, GROUPS], F32, kind="ExternalInput")
    env["selbc_d"] = nc.dram_tensor("selbc", [GROUPS, C], F32, kind="ExternalInput")
    env["out_d"] = nc.dram_tensor("out", [C, NQ], F32, kind="ExternalOutput")

    with tile.TileContext(nc) as tc:
        import contextlib

        with contextlib.ExitStack() as ctx:
            env["const"] = ctx.enter_context(tc.tile_pool(name="const", bufs=1))
            env["big"] = ctx.enter_context(tc.tile_pool(name="big", bufs=1))
            env["mid"] = ctx.enter_context(tc.tile_pool(name="mid", bufs=1))
            env["stats"] = ctx.enter_context(tc.tile_pool(name="stats", bufs=2))
            env["small"] = ctx.enter_context(tc.tile_pool(name="small", bufs=2))
            env["ppool"] = ctx.enter_context(tc.tile_pool(name="ppool", bufs=2))
            env["ps_work"] = ctx.enter_context(tc.tile_pool(name="ps_work", bufs=2, space="PSUM"))
            env["ps_v"] = ctx.enter_context(tc.tile_pool(name="ps_v", bufs=4, space="PSUM"))
            env["ps_rs"] = ctx.enter_context(tc.tile_pool(name="ps_rs", bufs=1, space="PSUM"))
            env["ps_small"] = ctx.enter_context(tc.tile_pool(name="ps_small", bufs=1, space="PSUM"))

            const = env["const"]
            ones8b = const.tile([128, 2, 128], F8, tag="ones8b")
            nc.vector.memset(ones8b, RSONE)
            env["ones8b"] = ones8b
            epst = const.tile([GROUPS, 1], F32, tag="epst")
            nc.vector.memset(epst, EPS)
            env["epst"] = epst
            ebias = const.tile([128, 1], F32, tag="ebias")
            nc.vector.memset(ebias, EBIAS)
            env["ebias"] = ebias
            dummy = const.tile([128, 1], F32, tag="dummy")
            env["dummy"] = dummy
            idf32 = const.tile([128, 16], F32, tag="idf32")
            nc.vector.memset(idf32, 1.0)
            env["idf32"] = idf32

            for rep in range(reps):
                body(nc, tc, env, first=(rep == 0))

    nc.compile()
    return nc


def body(nc, tc, env, first=True):
    big, mid, stats, small, ppool = (env[k] for k in ("big", "mid", "stats", "small", "ppool"))
    ps_work, ps_v, ps_rs, ps_small = (env[k] for k in ("ps_work", "ps_v", "ps_rs", "ps_small"))
    x_d, out_d = env["x_d"], env["out_d"]
    const = env["const"]
    ones8b, epst, ebias, dummy, idf32 = (
        env[k] for k in ("ones8b", "epst", "ebias", "dummy", "idf32"))
    AF = mybir.ActivationFunctionType
    wu_count = [0]

    def warmups(n, t_avail, lo=0, width=N):
        # full-array dummy bf16 matmuls on already-landed x pieces: 128-col
        # stationary + 256-col moving keeps all PE row/col groups active so
        # the HAM clock gate sees real occupancy, not 1-column trickles.
        for _ in range(n):
            i = wu_count[0]
            wu_count[0] += 1
            wu_ps = ps_rs.tile([128, 256], F32, tag="psrs", name=f"wu{i}")
            mo = lo + 128 + (i * 256) % (width - 384)
            nc.tensor.matmul(wu_ps, x_sb[:, t_avail, ds(lo, 128)],
                             x_sb[:, t_avail, ds(mo, 256)],
                             start=True, stop=True)

    def wu_paced(src):
        # small f32 matmul reading a just-produced DVE result: lands in the
        # PE queue right when the DVE finishes it, so sparse PE activity
        # tracks DVE progress through the stats phase (maintains un-throttle)
        i = wu_count[0]
        wu_count[0] += 1
        fw = src.shape[-1]
        wu_ps = ps_rs.tile([16, fw], F32, tag="psrs", name=f"wp{i}")
        nc.tensor.matmul(wu_ps, idf32[:src.shape[0], :], src,
                         start=True, stop=True)

    if first:
        def load_small_consts():
            for nm in ("bqk", "gnb"):
                sb = const.tile([128, T], F32, tag=nm, name=f"sb_{nm}")
                nc.gpsimd.dma_start(out=sb, in_=env[f"{nm}_d"][:, :])
                env[nm] = sb
            selred = const.tile([128, T, GROUPS], F32, tag="selred")
            nc.gpsimd.dma_start(out=selred, in_=env["selred_d"][:, :, :])
            env["selred"] = selred
            selbc = const.tile([GROUPS, C], F32, tag="selbc")
            nc.gpsimd.dma_start(out=selbc, in_=env["selbc_d"][:, :])
            env["selbc"] = selbc

        def load_weights():
            for nm in ("wqkT", "wvT", "wpT"):
                sb = const.tile([128, T, C], F8, tag=nm, name=f"sb_{nm}")
                dr_ = env[f"{nm}_d"]
                for t in range(T):
                    nc.gpsimd.dma_start(out=sb[:, t, :], in_=dr_[ts(t, 128), :])
                env[nm] = sb
            xq16 = mid.tile([128, T, NQ], BF16, tag="xq16")
            for t in range(T):
                nc.gpsimd.dma_start(out=xq16[:, t, :],
                                    in_=env["xq_d"][ts(t, 128), :])
            env["xq16"] = xq16

    # -------- load x + GroupNorm stats, pipelined per piece --------
    # x rides the two HW queues exclusively (everything else issues from
    # GpSimd). Tile halves alternate queues; the very first half-tile lands
    # in 512-col pieces so bn_stats starts ~3us earlier.
    x_sb = big.tile([128, T, N], BF16, tag="x")
    sts = []
    for t in range(T):
        st = stats.tile([128, 8, 6], F32, tag=f"bnstats{t}", bufs=1, name=f"st{t}")
        sts.append(st)

    psg = ps_small.tile([GROUPS, 2], F32, tag="pssmall")
    for t in range(T):
        for h in range(2):
            eng = nc.sync if h == 0 else nc.scalar
            lo = h * 2048
            eng.dma_start(out=x_sb[:, t, ds(lo, 2048)],
                          in_=x_d[ts(t, 128), ds(lo, 2048)])
            if (t, h) == (0, 0):
                warmups(16, 0, lo=0, width=2048)
            for sc in range(4):
                nc.vector.bn_stats(out=sts[t][:, 4 * h + sc, :],
                                   in_=x_sb[:, t, ds(lo + sc * 512, 512)])
                wu_paced(sts[t][:, 4 * h + sc, :])
        if t == 0 and first:
            load_small_consts()
        mv = stats.tile([128, 2], F32, tag=f"mv{t}", bufs=1, name=f"mv{t}")
        nc.vector.bn_aggr(out=mv, in_=sts[t])
        # mv := (mean, E[x^2]) ; E[x^2] = var + mean^2
        msq = stats.tile([128, 1], F32, tag="msq")
        nc.vector.tensor_mul(msq, mv[:, 0:1], mv[:, 0:1])
        nc.vector.tensor_add(mv[:, 1:2], mv[:, 1:2], msq)
        wu_paced(mv)
        nc.tensor.matmul(psg, env["selred"][:, t, :], mv,
                         start=(t == 0), stop=(t == T - 1))
    if first:
        load_weights()
    warmups(4, T - 1)

    wqkT, wvT, wpT = env["wqkT"], env["wvT"], env["wpT"]
    bqk, gnb, selbc, xq16 = env["bqk"], env["gnb"], env["selbc"], env["xq16"]

    # group scale/offset straight off the psg PSUM: rstd = 1/sqrt(var+eps),
    # offset = -mean*rstd  (gn_weight is folded into selbc host-side)
    psgs = small.tile([GROUPS, 2], F32, tag="psgs", bufs=1)
    nc.vector.tensor_copy(psgs, psg)
    gsc = small.tile([GROUPS, 2], F32, tag="gsc", bufs=1)
    gtmp = small.tile([GROUPS, 2], F32, tag="gtmp", bufs=1)
    nc.vector.tensor_mul(gtmp[:, 0:1], psgs[:, 0:1], psgs[:, 0:1])      # mean^2
    nc.vector.tensor_sub(gtmp[:, 1:2], psgs[:, 1:2], gtmp[:, 0:1])      # var
    wu_paced(gtmp)
    nc.scalar.activation(out=gsc[:, 0:1], in_=gtmp[:, 1:2], func=AF.Sqrt, bias=epst)
    # preload the Exp activation table NOW (ScalarE idle; its next real use
    # is the first scores exp, which must not eat the 1.5us table load)
    nc.scalar.activation(out=dummy, in_=ebias, func=AF.Exp, bias=0.0)
    nc.vector.reciprocal(gsc[:, 0:1], gsc[:, 0:1])                      # rstd
    nc.vector.tensor_mul(gsc[:, 1:2], psgs[:, 0:1], gsc[:, 0:1])
    nc.vector.tensor_scalar_mul(gsc[:, 1:2], gsc[:, 1:2], -1.0)        # offset
    wu_paced(gsc)

    # per-channel (scale, offset) via the gnw-folded broadcast matmul, then
    # GN-apply -> xn (fp8e4) on the DVE in 2048 pieces, query-half first
    scof = small.tile([128, T, 2], F32, tag="scof", bufs=1)
    xn = mid.tile([128, T, N], F8, tag="xn")
    for t in range(T):
        psbc = ps_small.tile([128, 2], F32, tag="pssmall", name=f"psbc{t}")
        nc.tensor.matmul(psbc, selbc[:, ts(t, 128)], gsc, start=True, stop=True)
        nc.vector.tensor_copy(scof[:, t, 0:1], psbc[:, 0:1])
        nc.vector.tensor_scalar_add(scof[:, t, 1:2], psbc[:, 1:2],
                                    gnb[:, t:t + 1])
    for t in range(T):
        nc.vector.tensor_scalar(
            out=xn[:, t, ds(0, 2048)], in0=x_sb[:, t, ds(0, 2048)],
            scalar1=scof[:, t, 0:1], scalar2=scof[:, t, 1:2],
            op0=MUL, op1=ADD,
        )
    for q in range(4, 8):
        for t in range(T):
            nc.vector.tensor_scalar(
                out=xn[:, t, ds(q * 512, 512)], in0=x_sb[:, t, ds(q * 512, 512)],
                scalar1=scof[:, t, 0:1], scalar2=scof[:, t, 1:2],
                op0=MUL, op1=ADD,
            )
    warmups(4, T - 1)

    # -------- B = Wqk xn_q + bqk  (fp8 DR; 2-bank PSUM, single wide evict) --
    b_sb = mid.tile([128, T, NQC, 512], F8, tag="b")
    for t_out in range(T):
        for nch in range(NQC):
            bps = ps_v.tile([128, 512], F32, tag="psv", name=f"bps{t_out}_{nch}")
            for g in range(T // 2):
                nc.tensor.matmul(bps,
                                 wqkT[:, 2 * g:2 * g + 2, ts(t_out, 128)],
                                 xn[:, 2 * g:2 * g + 2, ds(nch * 512, 512)],
                                 start=(g == 0), stop=(g == T // 2 - 1),
                                 perf_mode=DR)
            nc.vector.tensor_scalar_add(b_sb[:, t_out, nch, :], bps,
                                        bqk[:, t_out:t_out + 1])

    # -------- V^T (fp8 DR; 4-bank PSUM in ps_v, single 2048-wide evict) ----
    # keeps Wv's x16: v8 = 16*v, cancels against p8's 1/16 in AV exactly
    vT = big.tile([128, NKT, C], F8, tag="vT")
    for nkt in range(NKT):
        vps = ps_v.tile([128, 512], F32, tag="psv", name=f"vps{nkt}")
        for g in range(T // 2):
            nc.tensor.matmul(vps, xn[:, 2 * g:2 * g + 2, ts(nkt, 128)],
                             wvT[:, 2 * g:2 * g + 2, :],
                             start=(g == 0), stop=(g == T // 2 - 1),
                             perf_mode=DR)
        nc.vector.tensor_copy(vT[:, nkt, :], vps)

    # -------- attention + proj per query chunk --------
    xq32 = mid.tile([128, T, NQ], F32, tag="xq32")

    def pe_epilogue(ch):
        # proj (fp8 DR) ping-pongs the ps_small/ps_rs banks; the fused
        # scalar_tensor_tensor adds the host-precomputed residual base
        for t_out in range(T):
            pool = ps_small if t_out % 2 == 0 else ps_rs
            ps = pool.tile([128, 512], F32,
                           tag="pssmall" if t_out % 2 == 0 else "psrs",
                           name=f"prps{ch}_{t_out}")
            for g in range(T // 2):
                nc.tensor.matmul(ps, wpT[:, 2 * g:2 * g + 2, ts(t_out, 128)],
                                 o_sb[:, 2 * g:2 * g + 2, ds(ch * 512, 512)],
                                 start=(g == 0), stop=(g == T // 2 - 1),
                                 perf_mode=DR)
            nc.vector.scalar_tensor_tensor(
                out=xq32[:, t_out, ds(ch * 512, 512)], in0=ps, scalar=PRSC,
                in1=xq16[:, t_out, ds(ch * 512, 512)], op0=MUL, op1=ADD)
            nc.sync.dma_start(out=out_d[ts(t_out, 128), ds(ch * 512, 512)],
                              in_=xq32[:, t_out, ds(ch * 512, 512)])

    o_sb = mid.tile([128, T, NQ], F8, tag="o")
    for ch in range(NQC):
        if ch > 0:
            pe_epilogue(ch - 1)
        o_ps = [ps_v.tile([128, 512], F32, tag="psv", name=f"ops{ch}_{i}")
                for i in range(T)]
        rs_ps = ps_rs.tile([128, 512], F32, tag="psrs", name=f"rs{ch}")
        p8 = ppool.tile([128, NKT, 512], F8, tag="p")

        def emit_av(j):
            nc.tensor.matmul(rs_ps, ones8b, p8[:, 2 * j:2 * j + 2, :],
                             start=(j == 0), stop=(j == NKP - 1), perf_mode=DR)
            for tc_in in range(T):
                nc.tensor.matmul(o_ps[tc_in],
                                 vT[:, 2 * j:2 * j + 2, ts(tc_in, 128)],
                                 p8[:, 2 * j:2 * j + 2, :],
                                 start=(j == 0), stop=(j == NKP - 1), perf_mode=DR)

        prev = None
        for j in range(NKP):
            for h in range(2):
                nkt = 2 * j + h
                s_ps = ps_work.tile([128, 512], F32, tag="pswork")
                for g in range(T // 2):
                    nc.tensor.matmul(s_ps, xn[:, 2 * g:2 * g + 2, ts(nkt, 128)],
                                     b_sb[:, 2 * g:2 * g + 2, ch, :],
                                     start=(g == 0), stop=(g == T // 2 - 1),
                                     perf_mode=DR)
                nc.scalar.activation(out=p8[:, nkt, :], in_=s_ps,
                                     func=AF.Exp, scale=ESC, bias=ebias)
            if prev is not None:
                emit_av(prev)
            prev = j
        emit_av(prev)

        # rsinv = 256/rowsum (fast approx, ~18 bits); normalize-at-eviction
        # o8 = o_psum * rsinv ~ N(0, 6.7) — frees the accumulators for the
        # next chunk and leaves the epilogue a pure proj+add.
        rsinv = small.tile([128, 512], F32, tag="rsinv", name=f"rsinv{ch}")
        nc.vector.reciprocal_approx_fast(out=rsinv, in_=rs_ps)
        for tc_in in range(T):
            nc.vector.tensor_mul(o_sb[:, tc_in, ds(ch * 512, 512)],
                                 o_ps[tc_in], rsinv)

    pe_epilogue(NQC - 1)


_NC_CACHE = {}


def _get_nc(reps: int = 1):
    if reps not in _NC_CACHE:
        _NC_CACHE[reps] = build_nc(reps)
    return _NC_CACHE[reps]


def make_in_maps(x, gn_weight, gn_bias, qkv_weight, qkv_bias, proj_weight, proj_bias):
    x = np.asarray(x, np.float32)
    qkv_weight = np.asarray(qkv_weight, np.float32)
    proj_weight = np.asarray(proj_weight, np.float32)
    qkv_bias = np.asarray(qkv_bias, np.float32)
    proj_bias = np.asarray(proj_bias, np.float32)
    gn_weight = np.asarray(gn_weight, np.float32)
    gn_bias = np.asarray(gn_bias, np.float32)

    Wq, Wk, Wv = qkv_weight[0:C], qkv_weight[C:2 * C], qkv_weight[2 * C:3 * C]
    wqkT = np.ascontiguousarray((WS * (Wq.T @ Wk)).astype(ml_dtypes.float8_e4m3))
    wvT = np.ascontiguousarray((WS * Wv.T).astype(ml_dtypes.float8_e4m3))
    wpT = np.ascontiguousarray((WS * proj_weight.T).astype(ml_dtypes.float8_e4m3))

    def cols(v):  # [C] -> [128, T]
        return np.ascontiguousarray(v.reshape(T, 128).T.astype(np.float32))

    bqkv = WS * (Wk.T @ qkv_bias[0:C])
    fbv = proj_weight @ qkv_bias[2 * C:3 * C] + proj_bias

    p_idx = np.arange(128)
    selred = np.zeros((128, T, GROUPS), np.float32)
    selbc = np.zeros((GROUPS, C), np.float32)
    for t in range(T):
        g = t * (128 // GSIZE) + p_idx // GSIZE
        selred[p_idx, t, g] = 1.0 / GSIZE
        selbc[g, t * 128 + p_idx] = gn_weight[t * 128 + p_idx]

    shared = {
        "wqkT": wqkT, "wvT": wvT, "wpT": wpT,
        "bqk": cols(bqkv), "gnb": cols(gn_bias),
        "selred": selred, "selbc": selbc,
    }
    in_maps = []
    for core in range(8):
        b, qb = core // 4, core % 4
        xb = x[b].reshape(C, N)
        xr = np.roll(xb, -qb * NQ, axis=1)
        m = dict(shared)
        m["x"] = np.ascontiguousarray(xr.astype(ml_dtypes.bfloat16))
        m["xq"] = np.ascontiguousarray(
            (xr[:, 0:NQ] + fbv[:, None]).astype(ml_dtypes.bfloat16))
        in_maps.append(m)
    return in_maps


def kernel(x, gn_weight, gn_bias, qkv_weight, qkv_bias, proj_weight, proj_bias):
    nc = _get_nc(1)
    in_maps = make_in_maps(x, gn_weight, gn_bias, qkv_weight, qkv_bias,
                           proj_weight, proj_bias)
    res = run_bass_kernel_spmd(nc, in_maps, core_ids=list(range(8)))
    out = np.empty((B, C, N), np.float32)
    for core in range(8):
        b, qb = core // 4, core % 4
        out[b][:, qb * NQ:(qb + 1) * NQ] = res.results[core]["out"]
    return out.reshape(B, C, H, W, D)


# revision 20
# speedup vs baseline: 1.5892x; 1.0268x over previous
"""AttnBlock3D (GroupNorm + single-head self-attention + proj + residual) on 8 trn2 cores.

Sharding: core i handles (batch b = i//4, query-block qb = i%4) of 1024 query
positions. Attention is permutation-equivariant over positions, so each core
receives its batch's x with the position axis rolled so that its query block
occupies columns 0:1024. Each core computes GroupNorm + full V for its batch
(4x replicated within a batch group) and attention/proj/residual for its own
1024 query positions. No collectives.

Algebraic restructures (exact up to fp rounding):
  * Q and K projections are never materialized. With Wqk = Wk^T Wq and
    bqk = Wk^T bq (host-computed),
      scores^T[nk, nq] = xn[:, nk] . (Wqk xn[:, :1024] + bqk)[:, nq]
                         + (per-nq constants, which cancel in softmax).
  * Softmax skips the max subtraction; normalization is deferred and folded
    into the o eviction (o8 = o_psum * 256/rowsum), so the epilogue is just
    proj -> one fused (ps/4096 + xq) scalar_tensor_tensor + DMA.
  * The rowsum matmul uses an all-1/16 [128,2,128] stationary so the
    per-query sum lands broadcast across all 128 PSUM partitions; the
    reciprocal runs as a full-width [128,512] reciprocal_approx_fast
    (DVE InstReciprocal costs 4us; the approx is ~0.8us at 18 bits).
  * The residual base (x + folded proj/v bias) is precomputed ON THE HOST
    and shipped as a bf16 input, so no head DVE op touches it.
  * gn_weight is folded into the host-built selbc broadcast matrix.

fp8 DoubleRow everywhere: B, V, scores, AV+rowsum, proj all run as fp8e4
DoubleRow (2 fp8 MACs/cell/cycle), operands are [128, 2, free] planar
slices. Wqk/Wv/Wp are prescaled x16 on the host (fp8 subnormal avoidance);
exp gets scale SCALE/16 and bias -ln16; V keeps its x16 (cancels against
p8's 1/16 in AV exactly); the residual add's 1/4096 scalar absorbs
Wp's 16 and o8's 256/rowsum normalization. All exact in fp.

Engine assignment: ScalarE runs ONLY Sqrt (GN, once) and Exp (64 tiles; the
Exp ACT_TABLE_LOAD is prefetched by a dummy exp right after the Sqrt). ALL
PSUM evictions run on the DVE, as wide as PSUM allows: V evicts 4 banks
(2048) at a time from a [128,4,512] PSUM tile, B evicts 2 banks. V and B
accumulate in ps_v/ps_work so the scores stream's PSUM rotation never waits
on a V eviction. GpSimd issues every non-x DMA so the two HW queues carry
only x during the head.
"""

import math

import numpy as np
import ml_dtypes

import concourse.bass as bass
import concourse.tile as tile
from concourse import bacc, mybir
from concourse.bass import ds, ts
from concourse.bass_utils import run_bass_kernel_spmd

B, C, H, W, D = 2, 512, 16, 16, 16
N = H * W * D              # 4096 positions
NQ = N // 4                # 1024 query positions per core
T = C // 128               # 4 channel tiles
NKT = N // 128             # 32 key tiles
NKP = NKT // 2             # 16 key-tile pairs (DoubleRow granularity)
NQC = NQ // 512            # 2 query chunks of 512
GROUPS = 32
GSIZE = C // GROUPS        # 16 channels per group
EPS = 1e-6
SCALE = float(C) ** -0.5
WS = 16.0                  # host prescale on Wqk / Wv / Wp
ESC = SCALE / WS           # exp scale (absorbs Wqk's x16)
EBIAS = -math.log(16.0)    # exp bias: p < ~15; cancels via rowsum
RSONE = 1.0 / 4.0          # rowsum stationary -> rsinv = 64/rowsum
PRSC = 1.0 / 1024.0        # epilogue scalar: (16Wp)(16V)(64/rs) -> /1024

F32 = mybir.dt.float32
F16 = mybir.dt.float16
BF16 = mybir.dt.bfloat16
F8 = mybir.dt.float8e4
DR = mybir.MatmulPerfMode.DoubleRow
MUL = mybir.AluOpType.mult
ADD = mybir.AluOpType.add


def build_nc(reps: int = 1):
    nc = bacc.Bacc("TRN2", target_bir_lowering=False)

    env = {}
    env["x_d"] = nc.dram_tensor("x", [C, N], BF16, kind="ExternalInput")
    env["xq_d"] = nc.dram_tensor("xq", [C, NQ], BF16, kind="ExternalInput")
    env["wqkT_d"] = nc.dram_tensor("wqkT", [C, C], F8, kind="ExternalInput")
    env["wvT_d"] = nc.dram_tensor("wvT", [C, C], F8, kind="ExternalInput")
    env["wpT_d"] = nc.dram_tensor("wpT", [C, C], F8, kind="ExternalInput")
    env["bqk_d"] = nc.dram_tensor("bqk", [128, T], F32, kind="ExternalInput")
    env["gnb_d"] = nc.dram_tensor("gnb", [128, T], F32, kind="ExternalInput")
    env["selred_d"] = nc.dram_tensor("selred", [128, T, GROUPS], F32, kind="ExternalInput")
    env["selbc_d"] = nc.dram_tensor("selbc", [GROUPS, C], F32, kind="ExternalInput")
    env["out_d"] = nc.dram_tensor("out", [C, NQ], F32, kind="ExternalOutput")

    with tile.TileContext(nc) as tc:
        import contextlib

        with contextlib.ExitStack() as ctx:
            env["const"] = ctx.enter_context(tc.tile_pool(name="const", bufs=1))
            env["big"] = ctx.enter_context(tc.tile_pool(name="big", bufs=1))
            env["mid"] = ctx.enter_context(tc.tile_pool(name="mid", bufs=1))
            env["stats"] = ctx.enter_context(tc.tile_pool(name="stats", bufs=2))
            env["small"] = ctx.enter_context(tc.tile_pool(name="small", bufs=2))
            env["ppool"] = ctx.enter_context(tc.tile_pool(name="ppool", bufs=2))
            env["ps_work"] = ctx.enter_context(tc.tile_pool(name="ps_work", bufs=2, space="PSUM"))
            env["ps_v"] = ctx.enter_context(tc.tile_pool(name="ps_v", bufs=4, space="PSUM"))
            env["ps_rs"] = ctx.enter_context(tc.tile_pool(name="ps_rs", bufs=1, space="PSUM"))
            env["ps_small"] = ctx.enter_context(tc.tile_pool(name="ps_small", bufs=1, space="PSUM"))

            const = env["const"]
            ones8b = const.tile([128, 2, 128], F8, tag="ones8b")
            nc.vector.memset(ones8b, RSONE)
            env["ones8b"] = ones8b
            epst = const.tile([GROUPS, 1], F32, tag="epst")
            nc.vector.memset(epst, EPS)
            env["epst"] = epst
            ebias = const.tile([128, 1], F32, tag="ebias")
            nc.vector.memset(ebias, EBIAS)
            env["ebias"] = ebias
            dummy = const.tile([128, 1], F32, tag="dummy")
            env["dummy"] = dummy
            idf32 = const.tile([128, 16], F32, tag="idf32")
            nc.vector.memset(idf32, 1.0)
            env["idf32"] = idf32

            for rep in range(reps):
                body(nc, tc, env, first=(rep == 0))

    nc.compile()
    return nc


def body(nc, tc, env, first=True):
    big, mid, stats, small, ppool = (env[k] for k in ("big", "mid", "stats", "small", "ppool"))
    ps_work, ps_v, ps_rs, ps_small = (env[k] for k in ("ps_work", "ps_v", "ps_rs", "ps_small"))
    x_d, out_d = env["x_d"], env["out_d"]
    const = env["const"]
    ones8b, epst, ebias, dummy, idf32 = (
        env[k] for k in ("ones8b", "epst", "ebias", "dummy", "idf32"))
    AF = mybir.ActivationFunctionType
    wu_count = [0]

    def warmups(n, t_avail, lo=0, width=N):
        # full-array dummy bf16 matmuls on already-landed x pieces: 128-col
        # stationary + 256-col moving keeps all PE row/col groups active so
        # the HAM clock gate sees real occupancy, not 1-column trickles.
        for _ in range(n):
            i = wu_count[0]
            wu_count[0] += 1
            wu_ps = ps_rs.tile([128, 256], F32, tag="psrs", name=f"wu{i}")
            mo = lo + 128 + (i * 256) % (width - 384)
            nc.tensor.matmul(wu_ps, x_sb[:, t_avail, ds(lo, 128)],
                             x_sb[:, t_avail, ds(mo, 256)],
                             start=True, stop=True)

    def wu_paced(src):
        # small f32 matmul reading a just-produced DVE result: lands in the
        # PE queue right when the DVE finishes it, so sparse PE activity
        # tracks DVE progress through the stats phase (maintains un-throttle)
        i = wu_count[0]
        wu_count[0] += 1
        fw = src.shape[-1]
        wu_ps = ps_rs.tile([16, fw], F32, tag="psrs", name=f"wp{i}")
        nc.tensor.matmul(wu_ps, idf32[:src.shape[0], :], src,
                         start=True, stop=True)

    if first:
        def load_small_consts():
            for nm in ("bqk", "gnb"):
                sb = const.tile([128, T], F32, tag=nm, name=f"sb_{nm}")
                nc.gpsimd.dma_start(out=sb, in_=env[f"{nm}_d"][:, :])
                env[nm] = sb
            selred = const.tile([128, T, GROUPS], F32, tag="selred")
            nc.gpsimd.dma_start(out=selred, in_=env["selred_d"][:, :, :])
            env["selred"] = selred
            selbc = const.tile([GROUPS, C], F32, tag="selbc")
            nc.gpsimd.dma_start(out=selbc, in_=env["selbc_d"][:, :])
            env["selbc"] = selbc

        def load_weights():
            for nm in ("wqkT", "wvT", "wpT"):
                sb = const.tile([128, T, C], F8, tag=nm, name=f"sb_{nm}")
                dr_ = env[f"{nm}_d"]
                for t in range(T):
                    nc.gpsimd.dma_start(out=sb[:, t, :], in_=dr_[ts(t, 128), :])
                env[nm] = sb
            xq16 = mid.tile([128, T, NQ], BF16, tag="xq16")
            for t in range(T):
                nc.gpsimd.dma_start(out=xq16[:, t, :],
                                    in_=env["xq_d"][ts(t, 128), :])
            env["xq16"] = xq16

    # -------- load x + GroupNorm stats, pipelined per piece --------
    # x rides the two HW queues exclusively (everything else issues from
    # GpSimd). Tile halves alternate queues; the very first half-tile lands
    # in 512-col pieces so bn_stats starts ~3us earlier.
    x_sb = big.tile([128, T, N], BF16, tag="x")
    sts = []
    for t in range(T):
        st = stats.tile([128, 8, 6], F32, tag=f"bnstats{t}", bufs=1, name=f"st{t}")
        sts.append(st)

    psg = ps_small.tile([GROUPS, 2], F32, tag="pssmall")
    for t in range(T):
        for h in range(2):
            eng = nc.sync if h == 0 else nc.scalar
            lo = h * 2048
            eng.dma_start(out=x_sb[:, t, ds(lo, 2048)],
                          in_=x_d[ts(t, 128), ds(lo, 2048)])
            if (t, h) == (0, 0):
                warmups(16, 0, lo=0, width=2048)
            for sc in range(4):
                nc.vector.bn_stats(out=sts[t][:, 4 * h + sc, :],
                                   in_=x_sb[:, t, ds(lo + sc * 512, 512)])
                wu_paced(sts[t][:, 4 * h + sc, :])
        if t == 0 and first:
            load_small_consts()
        mv = stats.tile([128, 2], F32, tag=f"mv{t}", bufs=1, name=f"mv{t}")
        nc.vector.bn_aggr(out=mv, in_=sts[t])
        # mv := (mean, E[x^2]) ; E[x^2] = var + mean^2
        msq = stats.tile([128, 1], F32, tag="msq")
        nc.vector.tensor_mul(msq, mv[:, 0:1], mv[:, 0:1])
        nc.vector.tensor_add(mv[:, 1:2], mv[:, 1:2], msq)
        wu_paced(mv)
        nc.tensor.matmul(psg, env["selred"][:, t, :], mv,
                         start=(t == 0), stop=(t == T - 1))
    if first:
        load_weights()
    warmups(4, T - 1)

    wqkT, wvT, wpT = env["wqkT"], env["wvT"], env["wpT"]
    bqk, gnb, selbc, xq16 = env["bqk"], env["gnb"], env["selbc"], env["xq16"]

    # group scale/offset straight off the psg PSUM: rstd = 1/sqrt(var+eps),
    # offset = -mean*rstd  (gn_weight is folded into selbc host-side)
    psgs = small.tile([GROUPS, 2], F32, tag="psgs", bufs=1)
    nc.vector.tensor_copy(psgs, psg)
    gsc = small.tile([GROUPS, 2], F32, tag="gsc", bufs=1)
    gtmp = small.tile([GROUPS, 2], F32, tag="gtmp", bufs=1)
    nc.vector.tensor_mul(gtmp[:, 0:1], psgs[:, 0:1], psgs[:, 0:1])      # mean^2
    nc.vector.tensor_sub(gtmp[:, 1:2], psgs[:, 1:2], gtmp[:, 0:1])      # var
    wu_paced(gtmp)
    nc.scalar.activation(out=gsc[:, 0:1], in_=gtmp[:, 1:2], func=AF.Sqrt, bias=epst)
    nc.vector.reciprocal(gsc[:, 0:1], gsc[:, 0:1])                      # rstd
    nc.vector.tensor_mul(gsc[:, 1:2], psgs[:, 0:1], gsc[:, 0:1])
    nc.vector.tensor_scalar_mul(gsc[:, 1:2], gsc[:, 1:2], -1.0)        # offset
    wu_paced(gsc)

    # per-channel (scale, offset) via the gnw-folded broadcast matmul, then
    # GN-apply -> xn (fp8e4) on the DVE in 2048 pieces, query-half first
    scof = small.tile([128, T, 2], F32, tag="scof", bufs=1)
    xn = mid.tile([128, T, N], F8, tag="xn")
    psbc = ps_small.tile([128, T, 2], F32, tag="pssmall", name="psbc")
    for t in range(T):
        nc.tensor.matmul(psbc[:, t, :], selbc[:, ts(t, 128)], gsc,
                         start=True, stop=True)
    for t in range(T):
        nc.vector.tensor_copy(scof[:, t, 0:1], psbc[:, t, 0:1])
        nc.vector.tensor_scalar_add(scof[:, t, 1:2], psbc[:, t, 1:2],
                                    gnb[:, t:t + 1])
    for t in range(T):
        nc.vector.tensor_scalar(
            out=xn[:, t, ds(0, 2048)], in0=x_sb[:, t, ds(0, 2048)],
            scalar1=scof[:, t, 0:1], scalar2=scof[:, t, 1:2],
            op0=MUL, op1=ADD,
        )
    for q in range(4, 8):
        for t in range(T):
            nc.vector.tensor_scalar(
                out=xn[:, t, ds(q * 512, 512)], in0=x_sb[:, t, ds(q * 512, 512)],
                scalar1=scof[:, t, 0:1], scalar2=scof[:, t, 1:2],
                op0=MUL, op1=ADD,
            )
    warmups(4, T - 1)

    # -------- B = Wqk xn_q + bqk  (fp8 DR; 2-bank PSUM, single wide evict) --
    b_sb = mid.tile([128, T, NQC, 512], F8, tag="b")
    for t_out in range(T):
        for nch in range(NQC):
            bps = ps_v.tile([128, 512], F32, tag="psv", name=f"bps{t_out}_{nch}")
            for g in range(T // 2):
                nc.tensor.matmul(bps,
                                 wqkT[:, 2 * g:2 * g + 2, ts(t_out, 128)],
                                 xn[:, 2 * g:2 * g + 2, ds(nch * 512, 512)],
                                 start=(g == 0), stop=(g == T // 2 - 1),
                                 perf_mode=DR)
            nc.scalar.activation(out=b_sb[:, t_out, nch, :], in_=bps,
                                 func=AF.Identity, bias=bqk[:, t_out:t_out + 1])
    # preload the Exp activation table in ScalarE's idle window between the
    # b evictions and the first scores exp
    nc.scalar.activation(out=dummy, in_=ebias, func=AF.Exp, bias=0.0)

    # -------- V^T (fp8 DR; 4-bank PSUM in ps_v, single 2048-wide evict) ----
    # keeps Wv's x16: v8 = 16*v, cancels against p8's 1/16 in AV exactly
    vT = big.tile([128, NKT, C], F8, tag="vT")
    for nkt in range(NKT):
        vps = ps_v.tile([128, 512], F32, tag="psv", name=f"vps{nkt}")
        for g in range(T // 2):
            nc.tensor.matmul(vps, xn[:, 2 * g:2 * g + 2, ts(nkt, 128)],
                             wvT[:, 2 * g:2 * g + 2, :],
                             start=(g == 0), stop=(g == T // 2 - 1),
                             perf_mode=DR)
        nc.vector.tensor_copy(vT[:, nkt, :], vps)

    # -------- attention + proj per query chunk --------
    xq32 = mid.tile([128, T, NQ], F32, tag="xq32")

    def pe_epilogue(ch):
        # proj (fp8 DR) ping-pongs the ps_small/ps_rs banks; the fused
        # scalar_tensor_tensor adds the host-precomputed residual base
        for t_out in range(T):
            pool = ps_small if t_out % 2 == 0 else ps_rs
            ps = pool.tile([128, 512], F32,
                           tag="pssmall" if t_out % 2 == 0 else "psrs",
                           name=f"prps{ch}_{t_out}")
            for g in range(T // 2):
                nc.tensor.matmul(ps, wpT[:, 2 * g:2 * g + 2, ts(t_out, 128)],
                                 o_sb[:, 2 * g:2 * g + 2, ds(ch * 512, 512)],
                                 start=(g == 0), stop=(g == T // 2 - 1),
                                 perf_mode=DR)
            nc.vector.scalar_tensor_tensor(
                out=xq32[:, t_out, ds(ch * 512, 512)], in0=ps, scalar=PRSC,
                in1=xq16[:, t_out, ds(ch * 512, 512)], op0=MUL, op1=ADD)
            nc.sync.dma_start(out=out_d[ts(t_out, 128), ds(ch * 512, 512)],
                              in_=xq32[:, t_out, ds(ch * 512, 512)])

    o_sb = mid.tile([128, T, NQ], F8, tag="o")
    for ch in range(NQC):
        if ch > 0:
            pe_epilogue(ch - 1)
        o_ps = [ps_v.tile([128, 512], F32, tag="psv", name=f"ops{ch}_{i}")
                for i in range(T)]
        rs_ps = ps_rs.tile([128, 512], F32, tag="psrs", name=f"rs{ch}")
        p8 = ppool.tile([128, NKT, 512], F8, tag="p")

        def emit_av(j):
            nc.tensor.matmul(rs_ps, ones8b, p8[:, 2 * j:2 * j + 2, :],
                             start=(j == 0), stop=(j == NKP - 1), perf_mode=DR)
            for tc_in in range(T):
                nc.tensor.matmul(o_ps[tc_in],
                                 vT[:, 2 * j:2 * j + 2, ts(tc_in, 128)],
                                 p8[:, 2 * j:2 * j + 2, :],
                                 start=(j == 0), stop=(j == NKP - 1), perf_mode=DR)

        prev = None
        for j in range(NKP):
            for h in range(2):
                nkt = 2 * j + h
                s_ps = ps_work.tile([128, 512], F32, tag="pswork")
                for g in range(T // 2):
                    nc.tensor.matmul(s_ps, xn[:, 2 * g:2 * g + 2, ts(nkt, 128)],
                                     b_sb[:, 2 * g:2 * g + 2, ch, :],
                                     start=(g == 0), stop=(g == T // 2 - 1),
                                     perf_mode=DR)
                nc.scalar.activation(out=p8[:, nkt, :], in_=s_ps,
                                     func=AF.Exp, scale=ESC, bias=ebias)
            if prev is not None:
                emit_av(prev)
            prev = j
        emit_av(prev)

        # rsinv = 256/rowsum (fast approx, ~18 bits); normalize-at-eviction
        # o8 = o_psum * rsinv ~ N(0, 6.7) — frees the accumulators for the
        # next chunk and leaves the epilogue a pure proj+add.
        rsinv = small.tile([128, 512], F32, tag="rsinv", name=f"rsinv{ch}")
        nc.vector.reciprocal_approx_fast(out=rsinv, in_=rs_ps)
        for tc_in in range(T):
            nc.vector.tensor_mul(o_sb[:, tc_in, ds(ch * 512, 512)],
                                 o_ps[tc_in], rsinv)

    pe_epilogue(NQC - 1)


_NC_CACHE = {}


def _get_nc(reps: int = 1):
    if reps not in _NC_CACHE:
        _NC_CACHE[reps] = build_nc(reps)
    return _NC_CACHE[reps]


def make_in_maps(x, gn_weight, gn_bias, qkv_weight, qkv_bias, proj_weight, proj_bias):
    x = np.asarray(x, np.float32)
    qkv_weight = np.asarray(qkv_weight, np.float32)
    proj_weight = np.asarray(proj_weight, np.float32)
    qkv_bias = np.asarray(qkv_bias, np.float32)
    proj_bias = np.asarray(proj_bias, np.float32)
    gn_weight = np.asarray(gn_weight, np.float32)
    gn_bias = np.asarray(gn_bias, np.float32)

    Wq, Wk, Wv = qkv_weight[0:C], qkv_weight[C:2 * C], qkv_weight[2 * C:3 * C]
    wqkT = np.ascontiguousarray((WS * (Wq.T @ Wk)).astype(ml_dtypes.float8_e4m3))
    wvT = np.ascontiguousarray((WS * Wv.T).astype(ml_dtypes.float8_e4m3))
    wpT = np.ascontiguousarray((WS * proj_weight.T).astype(ml_dtypes.float8_e4m3))

    def cols(v):  # [C] -> [128, T]
        return np.ascontiguousarray(v.reshape(T, 128).T.astype(np.float32))

    bqkv = WS * (Wk.T @ qkv_bias[0:C])
    fbv = proj_weight @ qkv_bias[2 * C:3 * C] + proj_bias

    p_idx = np.arange(128)
    selred = np.zeros((128, T, GROUPS), np.float32)
    selbc = np.zeros((GROUPS, C), np.float32)
    for t in range(T):
        g = t * (128 // GSIZE) + p_idx // GSIZE
        selred[p_idx, t, g] = 1.0 / GSIZE
        selbc[g, t * 128 + p_idx] = gn_weight[t * 128 + p_idx]

    shared = {
        "wqkT": wqkT, "wvT": wvT, "wpT": wpT,
        "bqk": cols(bqkv), "gnb": cols(gn_bias),
        "selred": selred, "selbc": selbc,
    }
    in_maps = []
    for core in range(8):
        b, qb = core // 4, core % 4
        xb = x[b].reshape(C, N)
        xr = np.roll(xb, -qb * NQ, axis=1)
        m = dict(shared)
        m["x"] = np.ascontiguousarray(xr.astype(ml_dtypes.bfloat16))
        m["xq"] = np.ascontiguousarray(
            (xr[:, 0:NQ] + fbv[:, None]).astype(ml_dtypes.bfloat16))
        in_maps.append(m)
    return in_maps


def kernel(x, gn_weight, gn_bias, qkv_weight, qkv_bias, proj_weight, proj_bias):
    nc = _get_nc(1)
    in_maps = make_in_maps(x, gn_weight, gn_bias, qkv_weight, qkv_bias,
                           proj_weight, proj_bias)
    res = run_bass_kernel_spmd(nc, in_maps, core_ids=list(range(8)))
    out = np.empty((B, C, N), np.float32)
    for core in range(8):
        b, qb = core // 4, core % 4
        out[b][:, qb * NQ:(qb + 1) * NQ] = res.results[core]["out"]
    return out.reshape(B, C, H, W, D)
